# revision 24
# baseline (speedup 1.0000x reference)
"""Trainium2 Bass kernel for nn_Block_75840532513234 (dense transformer block).

Sharding: 8 cores; core c handles batch b = c//2 and head-half hh = c%2
(8 of 16 heads). The two cores of a pair all-reduce the c_proj partial sums
(row-sharded c_proj) after each of the two sub-blocks; the pair's even core's
output is used.

v2: bf16 matmul path end to end (HW runs f32r matmuls ~3x slower than bf16).
  - X master, weights, k/v features, attention, c_proj, collectives all bf16.
  - Host supplies xT (bf16) so branch 1 needs no on-chip transposes.
  - LN stats via one-pass bn_stats/bn_aggr; mean fold applied as a rank-1
    (negmu x wsum) matmul accumulated into the projection PSUM, so PSUM
    eviction is a single per-partition scale by rstd.
  - Softmax denominators via DVE reciprocal instead of Act Ln/Exp.
  - exp(scores) has no max subtraction (scores bounded ~10).
"""
import sys
import math

sys.path.insert(0, '/opt/trn_rl_repo')

import numpy as np
import ml_dtypes

# ---------------------------------------------------------------------------
# Patches for this container's walrus build: it allows only ONE sync-wait per
# instruction, while Tile attaches several (final drain; ldweights). Split the
# extras onto standalone single-wait EventSemaphore instructions.
# ---------------------------------------------------------------------------
import concourse.tile as tile
import concourse.bass as bass
from concourse import mybir
from concourse.vector_clock import ScopedClock

_ev_ctr = [0]


def _split_multi_waits(nc):
    for f in nc.m.functions:
        for bb in f.blocks:
            il = bb.instructions
            i = 0
            while i < len(il):
                inst = il[i]
                si = inst.sync_info
                if si is not None and si.on_wait and len(si.on_wait) > 1:
                    waits = list(si.on_wait)
                    si.on_wait.clear()
                    si.on_wait.append(waits[-1])
                    for w in waits[:-1]:
                        _ev_ctr[0] += 1
                        ev = mybir.InstEventSemaphore(
                            name=f"EVSPLIT-{_ev_ctr[0]}", ins=[], outs=[])
                        ev.engine = inst.engine
                        ev.sync_info = mybir.SyncInfo(on_wait=[], on_update=[])
                        ev.sync_info.on_wait.append(w)
                        il.insert(i, ev)
                        i += 1
                i += 1


def _patched_drain_and_barrier(self, tick_clock, wait_clock):
    nc = self.nc
    drain_inst = nc.sync.drain()
    wait_clock.add_sem_waits(
        drain_inst.ins, ScopedClock({None: tick_clock.global_clock}))
    nc.all_engine_barrier()
    popped = nc._tile_sem_poison_stack.pop()
    assert popped is self._sem_poison
    nc.clear_and_free_semaphores(list(self.sems.allocated().values()))
    nc.all_engine_barrier()


tile.TileContext._drain_and_barrier = _patched_drain_and_barrier

_orig_tile_exit = tile.TileContext.__exit__


def _patched_tile_exit(self, *a, **k):
    r = _orig_tile_exit(self, *a, **k)
    _split_multi_waits(self.nc)
    return r


tile.TileContext.__exit__ = _patched_tile_exit

# NTFF profile hook (trimmed image lacks antenv.axon_hooks).
import types as _types

if "antenv.axon_hooks" not in sys.modules:
    _m = _types.ModuleType("antenv.axon_hooks")
    _hook_store = [None]

    def _set_hook(h):
        _hook_store[0] = h

    def _get_hook():
        if _hook_store[0] is None:
            try:
                if '/root/.axon_site' not in sys.path:
                    sys.path.insert(0, '/root/.axon_site')
                from trn_agent_boot.trn_boot import _ntff_profile_via_ctypes
                _hook_store[0] = _ntff_profile_via_ctypes(
                    '/opt/axon/libaxon_pjrt.so')
            except Exception:
                return None
        return _hook_store[0]

    _m.set_axon_ntff_profile_hook = _set_hook
    _m.get_axon_ntff_profile_hook = _get_hook
    sys.modules["antenv.axon_hooks"] = _m
    import antenv as _antenv
    _antenv.axon_hooks = _m

from concourse.bass_utils import run_bass_kernel_spmd  # noqa: E402

# ---------------------------------------------------------------------------
# Problem constants (hardcoded per the grading contract)
# ---------------------------------------------------------------------------
B, T, C, NH = 4, 1024, 1024, 16
HS = C // NH              # 64
NHL = 8                   # heads per core
CL = NHL * HS             # 512 local channels
EXP_SCALING = 10.0
KSCALE_MAX = float(np.log(2.0 ** 16 - 1))
N_CORES = 8
GROUPS = [[0, 1], [2, 3], [4, 5], [6, 7]]
# pm ReduceScatter chunks: staged-row start, rows, pm_out offset; keyed by
# the t-block whose staging completes the chunk (last two are single-block
# so the tail collective is small)
PM_FIRE = {1: (0, 256, 0), 3: (256, 256, 128), 5: (512, 256, 256),
           6: (768, 128, 384), 7: (896, 128, 448)}
PM_CHUNKS = [(0, 256, 0), (256, 256, 128), (512, 256, 256),
             (768, 128, 384), (896, 128, 448)]

f32 = mybir.dt.float32
bf16 = mybir.dt.bfloat16
AF = mybir.ActivationFunctionType
ALU = mybir.AluOpType
AXL = mybir.AxisListType

NTB = T // 128            # 8 t-blocks
NCB = C // 128            # 8 c-blocks

nbf = ml_dtypes.bfloat16


def _build_program():
    nc = bass.Bass(num_devices=N_CORES)

    # ---- I/O (bf16 unless noted) ----
    x_in = nc.dram_tensor("x", [T, C], bf16, kind="ExternalInput")
    xt_in = nc.dram_tensor("xt", [C, T], bf16, kind="ExternalInput")
    wka_in = nc.dram_tensor("wka", [C, CL], bf16, kind="ExternalInput")
    wva_in = nc.dram_tensor("wva", [C, CL], bf16, kind="ExternalInput")
    wkm_in = nc.dram_tensor("wkm", [C, CL], bf16, kind="ExternalInput")
    cpa_in = nc.dram_tensor("cpa", [CL, C], bf16, kind="ExternalInput")
    cpm_in = nc.dram_tensor("cpm", [CL, C], bf16, kind="ExternalInput")
    # column-sum rows of the (LN-folded) projection weights, for the mu fold
    wkar_in = nc.dram_tensor("wkar", [1, CL], bf16, kind="ExternalInput")
    wvar_in = nc.dram_tensor("wvar", [1, CL], bf16, kind="ExternalInput")
    wkmr_in = nc.dram_tensor("wkmr", [1, CL], bf16, kind="ExternalInput")
    dmata_in = nc.dram_tensor("dmata", [128, NHL * 256], bf16, kind="ExternalInput")
    dmatm_in = nc.dram_tensor("dmatm", [128, NHL * 256], bf16, kind="ExternalInput")
    sveca_in = nc.dram_tensor("sveca", [128, NHL], f32, kind="ExternalInput")
    svecm_in = nc.dram_tensor("svecm", [128, NHL], f32, kind="ExternalInput")
    coefb_in = nc.dram_tensor("coefb", [128, CL], bf16, kind="ExternalInput")
    vs_in = nc.dram_tensor("vs", [128, NHL], f32, kind="ExternalInput")
    pkt_in = nc.dram_tensor("pkt", [128, 4096], bf16, kind="ExternalInput")
    pvo_in = nc.dram_tensor("pvo", [128, NHL * 8 * 66], bf16, kind="ExternalInput")
    ident_in = nc.dram_tensor("ident", [128, 128], bf16, kind="ExternalInput")
    maskt_in = nc.dram_tensor("maskt", [128, 4 * 512], bf16, kind="ExternalInput")
    ones1_in = nc.dram_tensor("ones1", [1, 64], bf16, kind="ExternalInput")
    onesr_in = nc.dram_tensor("onesr", [1, 512], bf16, kind="ExternalInput")
    onescol_in = nc.dram_tensor("onescol", [128, 1], bf16, kind="ExternalInput")
    vpad_in = nc.dram_tensor("vpad", [128, 16], bf16, kind="ExternalInput")
    zcol_in = nc.dram_tensor("zcol", [128, 1], bf16, kind="ExternalInput")
    epsv_in = nc.dram_tensor("epsv", [128, 1], f32, kind="ExternalInput")
    smat_in = nc.dram_tensor("smat", [128, 256], bf16, kind="ExternalInput")

    xp_out = nc.dram_tensor("xp", [T, C], bf16, kind="ExternalOutput")
    pm_out = nc.dram_tensor("pm", [512, C], bf16, kind="ExternalOutput")

    cc1_in = nc.dram_tensor("cc1_in", [T, C], bf16)
    cc1_out = nc.dram_tensor("cc1_out", [T, C], bf16)
    cc2_in = nc.dram_tensor("cc2_in", [T, C], bf16)
    cc2_out = nc.dram_tensor("cc2_out", [512, C], bf16)

    with tile.TileContext(nc) as tc:
        # ---------------- persistent pools ----------------
        with tc.tile_pool(name="persist", bufs=1) as pp, \
             tc.tile_pool(name="work", bufs=1) as wp:
            # x and xT first: they gate the first projection group
            X = wp.tile([128, NTB * 1024], bf16, tag="X")
            for tb in range(NTB):
                nc.sync.dma_start(
                    X[:, tb * 1024:(tb + 1) * 1024],
                    x_in[tb * 128:(tb + 1) * 128, :])
            XT = wp.tile([128, NCB * 1024], bf16, tag="XT")
            for cb in range(NCB):
                nc.gpsimd.dma_start(
                    XT[:, cb * 1024:(cb + 1) * 1024],
                    xt_in[cb * 128:(cb + 1) * 128, :])

            # constants
            ident = pp.tile([128, 128], bf16, tag="ident")
            nc.sync.dma_start(ident[:], ident_in[:])
            sveca = pp.tile([128, NHL], f32, tag="sveca")
            nc.sync.dma_start(sveca[:], sveca_in[:])
            svecm = pp.tile([128, NHL], f32, tag="svecm")
            nc.sync.dma_start(svecm[:], svecm_in[:])
            coefb = pp.tile([128, CL], bf16, tag="coefb")
            nc.sync.dma_start(coefb[:], coefb_in[:])
            vsv = pp.tile([128, NHL], f32, tag="vsv")
            nc.sync.dma_start(vsv[:], vs_in[:])
            ones1 = pp.tile([1, 64], bf16, tag="ones1")
            nc.sync.dma_start(ones1[:], ones1_in[:])
            onesr = pp.tile([1, 512], bf16, tag="onesr")
            nc.sync.dma_start(onesr[:], onesr_in[:])
            onescol = pp.tile([128, 1], bf16, tag="onescol")
            nc.sync.dma_start(onescol[:], onescol_in[:])
            vpad = pp.tile([128, 16], bf16, tag="vpad")
            nc.sync.dma_start(vpad[:], vpad_in[:])
            zcol = pp.tile([128, 1], bf16, tag="zcol")
            nc.sync.dma_start(zcol[:], zcol_in[:])
            epsv = pp.tile([128, 1], f32, tag="epsv")
            nc.sync.dma_start(epsv[:], epsv_in[:])
            smat = pp.tile([128, 256], bf16, tag="smat")
            nc.sync.dma_start(smat[:], smat_in[:])
            maskt = pp.tile([128, 2048], bf16, tag="maskt")
            nc.sync.dma_start(maskt[:], maskt_in[:])
            pvo = pp.tile([128, NHL * 8 * 66], bf16, tag="pvo")
            nc.gpsimd.dma_start(pvo[:], pvo_in[:])
            wrow = {}
            for nm, src in (("wka", wkar_in), ("wva", wvar_in),
                            ("wkm", wkmr_in)):
                wrow[nm] = pp.tile([1, CL], bf16, name=f"wr_{nm}",
                                   tag=f"wr_{nm}")
                nc.sync.dma_start(wrow[nm][:], src[:])

            def branch(branch_id, wk_in, wkr, dmat_in, svec, cc_in_t,
                       wv_in=None, wvr=None, cp_in=None, cc_out_t=None,
                       rstd_pre=None):
                """Emit one sub-block (context: with v; persistent: without).

                ctx: AllReduce partial c_proj sums, add residual into X.
                pm: ReduceScatter straight into pm_out (host stitches).
                """
                is_ctx = wv_in is not None

                # ---- LN stats: var/rstd via one-pass bn_stats (column
                # form); -mean as a [1, T] row via an ones-column matmul on
                # XT, emitted per 512-t half inside the projection loop so
                # branch 2 can start before all residual chunks land ----
                with tc.tile_pool(name=f"st{branch_id}", bufs=NTB) as sp:
                    if rstd_pre is None:
                        rstd = []
                        with tc.tile_pool(name=f"sttmp{branch_id}",
                                          bufs=2) as stp:
                            for tb in range(NTB):
                                xs = X[:, tb * 1024:(tb + 1) * 1024]
                                bn6 = stp.tile([128, 12], f32, tag="bn6")
                                nc.vector.bn_stats(bn6[:, 0:6], xs[:, 0:512])
                                nc.vector.bn_stats(bn6[:, 6:12],
                                                   xs[:, 512:1024])
                                ms = stp.tile([128, 2], f32, tag="ms")
                                nc.vector.bn_aggr(
                                    ms[:],
                                    bn6[:].rearrange("p (a s) -> p a s", a=2))
                                lv = stp.tile([128, 1], f32, tag="lv")
                                nc.scalar.activation(lv[:], ms[:, 1:2], AF.Ln,
                                                     bias=epsv[:])
                                rs = sp.tile([128, 1], f32, tag="rstd")
                                nc.scalar.activation(rs[:], lv[:], AF.Exp,
                                                     scale=-0.5)
                                rstd.append(rs)
                    else:
                        rstd = rstd_pre
                    negmuT = sp.tile([1, 1024], bf16, tag="negmuT")

                    # ---- projections (xT stationary; mu fold via matmul) ----
                    kraw = wp.tile([128, NTB * CL], bf16, tag="kraw")
                    vraw = None
                    if is_ctx:
                        vraw = wp.tile([128, NTB * CL], bf16, tag="vraw")
                    tgroups = [(0, 3), (3, 6), (6, 8)]
                    with tc.tile_pool(name=f"pj{branch_id}", bufs=1,
                                      space="PSUM") as pjp, \
                         tc.tile_pool(name=f"stps{branch_id}", bufs=2,
                                      space="PSUM") as stpp, \
                         tc.tile_pool(name=f"w{branch_id}", bufs=4) as wpool:

                        def emit_psmu(q):
                            psmu = stpp.tile([1, 512], f32, tag="psmu")
                            for cb in range(NCB):
                                nc.tensor.matmul(
                                    psmu[:], onescol[:],
                                    XT[:, cb * 1024 + q * 512:
                                       cb * 1024 + q * 512 + 512],
                                    start=(cb == 0), stop=(cb == NCB - 1))
                            nc.vector.tensor_scalar_mul(
                                negmuT[0:1, q * 512:(q + 1) * 512],
                                psmu[:], -1.0 / C)

                        for gi, (g0, g1) in enumerate(tgroups):
                            if gi == 0:
                                emit_psmu(0)
                            elif gi == 1:
                                emit_psmu(1)
                            gn = g1 - g0
                            psk = [pjp.tile([128, CL], f32,
                                            name=f"psk{g0}_{i}",
                                            tag=f"psk{i}")
                                   for i in range(gn)]
                            psv = [pjp.tile([128, CL], f32,
                                            name=f"psv{g0}_{i}",
                                            tag=f"psv{i}")
                                   for i in range(gn)] if is_ctx else None
                            for cb in range(NCB):
                                wk_c = wpool.tile([128, CL], bf16, tag="wk")
                                nc.gpsimd.dma_start(
                                    wk_c[:], wk_in[cb * 128:(cb + 1) * 128, :])
                                wv_c = None
                                if is_ctx:
                                    wv_c = wpool.tile([128, CL], bf16,
                                                      tag="wv")
                                    nc.gpsimd.dma_start(
                                        wv_c[:],
                                        wv_in[cb * 128:(cb + 1) * 128, :])
                                for i, tb in enumerate(range(g0, g1)):
                                    lhs = XT[:, cb * 1024 + tb * 128:
                                             cb * 1024 + tb * 128 + 128]
                                    nc.tensor.matmul(
                                        psk[i][:], lhs, wk_c[:],
                                        start=(cb == 0), stop=False)
                                    if is_ctx:
                                        nc.tensor.matmul(
                                            psv[i][:], lhs, wv_c[:],
                                            start=(cb == 0), stop=False)
                            # mu fold: psum += negmu[t] * colsum(W)[ch]
                            for i, tb in enumerate(range(g0, g1)):
                                nc.tensor.matmul(
                                    psk[i][:],
                                    negmuT[0:1, tb * 128:(tb + 1) * 128],
                                    wkr[:], start=False, stop=True)
                                if is_ctx:
                                    nc.tensor.matmul(
                                        psv[i][:],
                                        negmuT[0:1, tb * 128:(tb + 1) * 128],
                                        wvr[:], start=False, stop=True)
                            # evict: single per-partition scale by rstd
                            for i, tb in enumerate(range(g0, g1)):
                                nc.vector.tensor_scalar_mul(
                                    kraw[:, tb * CL:(tb + 1) * CL],
                                    psk[i][:], rstd[tb][:])
                                if is_ctx:
                                    nc.vector.tensor_scalar_mul(
                                        vraw[:, tb * CL:(tb + 1) * CL],
                                        psv[i][:], rstd[tb][:])

                # ---- v path (context only) ----
                vnorm = None
                if is_ctx:
                    vnorm = wp.tile([128, NTB * 528], bf16, tag="vnorm")
                    with tc.tile_pool(name="vtmp", bufs=3) as vtp, \
                         tc.tile_pool(name="vps", bufs=2,
                                      space="PSUM") as vpsp, \
                         tc.tile_pool(name="vst", bufs=3) as vsp:
                        for tb in range(NTB):
                            vr = vraw[:, tb * CL:(tb + 1) * CL]
                            # v_shift on the PE: S@v (+ E@v_next for row 127)
                            pvs = vpsp.tile([128, CL], f32, tag="pvs")
                            nc.tensor.matmul(
                                pvs[:], smat[:, 0:128], vr,
                                start=True, stop=(tb == NTB - 1))
                            if tb < NTB - 1:
                                nc.tensor.matmul(
                                    pvs[:], smat[:, 128:256],
                                    vraw[:, (tb + 1) * CL:(tb + 2) * CL],
                                    start=False, stop=True)
                            vsh = vtp.tile([128, CL], bf16, tag="vsh")
                            # vmix = vraw + (1-coef)*(vsh - vraw)
                            nc.vector.tensor_tensor(vsh[:], pvs[:], vr,
                                                    ALU.subtract)
                            nc.vector.tensor_tensor(vsh[:], vsh[:], coefb[:],
                                                    ALU.mult)
                            nc.vector.tensor_tensor(vsh[:], vsh[:], vr,
                                                    ALU.add)
                            sq = vtp.tile([128, CL], bf16, tag="vsq")
                            nc.scalar.square(sq[:], vsh[:])
                            ssq = vsp.tile([128, NHL], f32, tag="vssq")
                            nc.vector.reduce_sum(
                                ssq[:], sq[:].rearrange("p (h d) -> p h d",
                                                        h=NHL), axis=AXL.X)
                            lnv = vsp.tile([128, NHL], f32, tag="vlnv")
                            nc.scalar.activation(lnv[:], ssq[:], AF.Ln)
                            rn = vsp.tile([128, NHL], f32, tag="vrn")
                            nc.scalar.activation(rn[:], lnv[:], AF.Exp,
                                                 scale=-0.5)
                            rns = vsp.tile([128, NHL], f32, tag="vrns")
                            nc.vector.tensor_tensor(rns[:], rn[:], vsv[:],
                                                    ALU.mult)
                            rns_b = rns[:].unsqueeze(2).broadcast_to(
                                (128, NHL, HS))
                            vslice = vnorm[:, tb * 528:(tb + 1) * 528]
                            nc.vector.tensor_tensor(
                                vslice.rearrange("p (h c) -> p h c",
                                                 h=NHL)[:, :, 0:64],
                                vsh[:].rearrange("p (h d) -> p h d", h=NHL),
                                rns_b, ALU.mult)
                            nc.sync.dma_start(
                                vslice.rearrange("p (h c) -> p h c",
                                                 h=NHL)[:, :, 64:66],
                                vpad[:].rearrange("p (h t) -> p h t", h=NHL))

                # ---- LeakyAvg + normalize + transpose -> ktall ----
                dmat = wp.tile([128, NHL * 256], bf16, tag="dmat")
                nc.gpsimd.dma_start(dmat[:], dmat_in[:])
                ktall = wp.tile([128, 4096], bf16, tag="ktall")
                with tc.tile_pool(name=f"lv{branch_id}", bufs=2,
                                  space="PSUM") as lvp, \
                     tc.tile_pool(name=f"ltr{branch_id}", bufs=2,
                                  space="PSUM") as ltp, \
                     tc.tile_pool(name=f"le{branch_id}", bufs=3) as lep, \
                     tc.tile_pool(name=f"ls{branch_id}", bufs=4) as lsp:
                    ktr_pending = []

                    def emit_ktr():
                        h0, kf0 = ktr_pending.pop(0)
                        pb0 = (h0 % 2) * 64
                        fb0 = (h0 // 2) * 1024
                        for half in range(2):
                            ptr = ltp.tile([64, 512], bf16, tag="ktr")
                            for q in range(4):
                                blk = half * 4 + q
                                nc.tensor.transpose(
                                    ptr[:, q * 128:(q + 1) * 128],
                                    kf0[:, blk * 64:(blk + 1) * 64],
                                    ident[:])
                            nc.vector.tensor_copy(
                                ktall[pb0:pb0 + 64,
                                      fb0 + half * 512:fb0 + half * 512
                                      + 512],
                                ptr[:])

                    for h in range(NHL):
                        pl = lvp.tile([128, CL], f32, tag="pl")
                        kview = kraw[:].rearrange("p (b r) -> p b r", r=CL)
                        rhs_all = kview[:, :, h * 64:h * 64 + 64]
                        nc.tensor.matmul(
                            pl[:], dmat[:, h * 256:h * 256 + 128],
                            rhs_all, start=True, stop=False)
                        rhs_prev = kview[:, 0:7, h * 64:h * 64 + 64]
                        nc.tensor.matmul(
                            pl[:, 64:512], dmat[:, h * 256 + 128:h * 256 + 256],
                            rhs_prev, start=False, stop=True)
                        lsq = lep.tile([128, CL], bf16, tag="lsq")
                        nc.scalar.square(lsq[:], pl[:])
                        ssq = lsp.tile([128, 8], f32, tag="lssq")
                        nc.vector.reduce_sum(
                            ssq[:], lsq[:].rearrange("p (b d) -> p b d",
                                                     d=64), axis=AXL.X)
                        lnv = lsp.tile([128, 8], f32, tag="llnv")
                        nc.scalar.activation(lnv[:], ssq[:], AF.Ln)
                        rn = lsp.tile([128, 8], f32, tag="lrn")
                        nc.scalar.activation(rn[:], lnv[:], AF.Exp, scale=-0.5)
                        rns = lsp.tile([128, 8], f32, tag="lrns")
                        nc.vector.tensor_scalar_mul(rns[:], rn[:],
                                                    svec[:, h:h + 1])
                        kfeat = lep.tile([128, CL], bf16, tag="kfeat")
                        rb = rns[:].unsqueeze(2).broadcast_to((128, 8, 64))
                        nc.vector.tensor_tensor(
                            kfeat[:].rearrange("p (b d) -> p b d", d=64),
                            pl[:].rearrange("p (b d) -> p b d", d=64),
                            rb, ALU.mult)
                        # transpose 8 blocks of (128,64) -> (64,128); emitted
                        # one head late so the PE isn't waiting on this
                        # head's normalize chain
                        ktr_pending.append((h, kfeat))
                        if len(ktr_pending) > 1:
                            emit_ktr()
                    while ktr_pending:
                        emit_ktr()

                # ---- attention (qc-major) + c_proj + collectives ----
                # For each query half: all heads' scores/exp/AV, then the
                # c_proj rows of that half and their collective chunks, so
                # AllReduce/ReduceScatter overlap the other half's attention.
                # Scores go 2 key-blocks per PSUM tile (exp on [128,1024]);
                # AV of pair j is emitted after the scores of pair j+1.
                ytall = wp.tile([128, 4096], bf16, tag="ytall")
                with tc.tile_pool(name=f"ap{branch_id}", bufs=1,
                                  space="PSUM") as app, \
                     tc.tile_pool(name=f"at{branch_id}", bufs=4) as atp, \
                     tc.tile_pool(name=f"ar{branch_id}", bufs=4) as arp, \
                     tc.tile_pool(name=f"cw{branch_id}", bufs=2) as cwp, \
                     tc.tile_pool(name=f"cs{branch_id}", bufs=3) as csp:
                    cpw = [None] * 4
                    for cb in range(4):
                        cpw[cb] = cwp.tile([128, 1024], bf16,
                                           name=f"cpw{cb}", tag=f"cpw{cb}")
                        nc.gpsimd.dma_start(
                            cpw[cb][:], cp_in[cb * 128:(cb + 1) * 128, :])
                    pending = []
                    ctr = {"ps": 0, "py": 0}

                    def flush_denom():
                        h, qc, py = pending.pop(0)
                        pbase = (h % 2) * 64
                        fbase = (h // 2) * 1024
                        lrow = arp.tile([1, 512], f32, tag="lrow")
                        nc.scalar.activation(lrow[:], py[64:65, :], AF.Ln)
                        rrowb = arp.tile([1, 512], bf16, tag="rrowb")
                        nc.scalar.activation(rrowb[:], lrow[:], AF.Exp,
                                             scale=-1.0)
                        pb = app.tile([64, 512], f32, tag="pb")
                        nc.tensor.matmul(pb[:], ones1[:], rrowb[:],
                                         start=True, stop=True)
                        bcs = atp.tile([64, 512], bf16, tag="bcs")
                        nc.vector.tensor_copy(bcs[:], pb[:])
                        nc.vector.tensor_tensor(
                            ytall[pbase:pbase + 64,
                                  fbase + qc * 512:fbase + qc * 512 + 512],
                            py[0:64, :], bcs[:], ALU.mult)
                        if is_ctx and qc == 0:
                            # zero out the t=0 column (query 0 has no keys)
                            nc.sync.dma_start(
                                ytall[pbase:pbase + 64, fbase:fbase + 1],
                                zcol[0:64, :])

                    for qc in range(2):
                        for h in range(NHL):
                            pbase = (h % 2) * 64
                            fbase = (h // 2) * 1024
                            kt_h = ktall[pbase:pbase + 64,
                                         fbase:fbase + 1024]
                            py = app.tile([66, 512], f32,
                                          name=f"py{qc}_{h}",
                                          tag=f"py{ctr['py'] % 2}")
                            ctr['py'] += 1
                            njb = 4 if (is_ctx and qc == 0) else 8
                            npair = njb // 2
                            avq = []

                            def emit_avs():
                                jb0, att0 = avq.pop(0)
                                for k in range(2):
                                    jb = jb0 + k
                                    if is_ctx:
                                        lhs_v = vnorm[
                                            :, jb * 528 + h * 66:
                                            jb * 528 + (h + 1) * 66]
                                    else:
                                        lhs_v = pvo[:, h * 528 + jb * 66:
                                                    h * 528 + (jb + 1) * 66]
                                    nc.tensor.matmul(
                                        py[:], lhs_v,
                                        att0[:, k * 512:(k + 1) * 512],
                                        start=(jb == 0),
                                        stop=(jb == njb - 1))

                            for jp in range(npair):
                                ps = app.tile([128, 1024], f32,
                                              name=f"ps{qc}_{h}_{jp}",
                                              tag=f"ps{ctr['ps'] % 2}")
                                ctr['ps'] += 1
                                for k in range(2):
                                    jb = jp * 2 + k
                                    if is_ctx:
                                        lhs_sc = kt_h[:, jb * 128:
                                                      (jb + 1) * 128]
                                    else:
                                        lhs_sc = pktall[
                                            pbase:pbase + 64,
                                            fbase + jb * 128:
                                            fbase + (jb + 1) * 128]
                                    nc.tensor.matmul(
                                        ps[:, k * 512:(k + 1) * 512], lhs_sc,
                                        kt_h[:, qc * 512:(qc + 1) * 512],
                                        start=True, stop=True)
                                att = atp.tile([128, 1024], bf16, tag="att")
                                nc.scalar.activation(att[:], ps[:], AF.Exp)
                                r = jp * 2 - qc * 4
                                if is_ctx and r >= 0:
                                    nc.vector.tensor_tensor(
                                        att[:], att[:],
                                        maskt[:, r * 512:r * 512 + 1024],
                                        ALU.mult)
                                avq.append((jp * 2, att))
                                if jp > 0:
                                    emit_avs()
                            emit_avs()
                            pending.append((h, qc, py))
                            if len(pending) > 1:
                                flush_denom()
                        while pending:
                            flush_denom()
                        # c_proj rows of this query half + collective chunks
                        for tb in range(qc * 4, qc * 4 + 4):
                            stage = csp.tile([128, 1024], bf16, tag="cstage")
                            for co in range(2):
                                pc = app.tile([128, 512], f32, tag="pc")
                                for cb in range(4):
                                    nc.tensor.matmul(
                                        pc[:],
                                        ytall[:, cb * 1024 + tb * 128:
                                              cb * 1024 + tb * 128 + 128],
                                        cpw[cb][:, co * 512:(co + 1) * 512],
                                        start=(cb == 0), stop=(cb == 3))
                                nc.vector.tensor_copy(
                                    stage[:, co * 512:(co + 1) * 512], pc[:])
                            nc.sync.dma_start(
                                cc_in_t[tb * 128:(tb + 1) * 128, :],
                                stage[:])
                            if is_ctx:
                                if tb % 2 == 1:
                                    ch = tb // 2
                                    nc.gpsimd.collective_compute(
                                        "AllReduce", ALU.add,
                                        replica_groups=GROUPS,
                                        ins=[cc_in_t[ch * 256:
                                                     (ch + 1) * 256, :]],
                                        outs=[cc_out_t[ch * 256:
                                                       (ch + 1) * 256, :]])
                            elif tb in PM_FIRE:
                                t0r, nr, po = PM_FIRE[tb]
                                hn = nr // 2
                                nc.gpsimd.collective_compute(
                                    "ReduceScatter", ALU.add,
                                    replica_groups=GROUPS,
                                    ins=[cc_in_t[t0r:t0r + nr, :]],
                                    outs=[cc2_out[po:po + hn, :]])
                                nc.gpsimd.dma_start(
                                    pm_out[po:po + hn, :],
                                    cc2_out[po:po + hn, :])
            # -------- context branch --------
            pktall = None
            branch(0, wka_in, wrow["wka"], dmata_in, sveca, cc1_in,
                   wv_in=wva_in, wvr=wrow["wva"], cp_in=cpa_in,
                   cc_out_t=cc1_out)

            # ---- ctx residual + xp + branch-2 LN stats + x'^T, per
            # t-block as its AllReduce chunk lands; transposes ride the DMA
            # xbar so the PE queue stays free across the transition ----
            rstd2 = []
            with tc.tile_pool(name="tr", bufs=NTB) as trp, \
                 tc.tile_pool(name="trt", bufs=3) as trtp:
                for tb in range(NTB):
                    xs = X[:, tb * 1024:(tb + 1) * 1024]
                    back = trtp.tile([128, 1024], bf16, tag="cback")
                    nc.sync.dma_start(
                        back[:], cc1_out[tb * 128:(tb + 1) * 128, :])
                    nc.vector.tensor_tensor(xs, xs, back[:], ALU.add)
                    nc.sync.dma_start(
                        xp_out[tb * 128:(tb + 1) * 128, :], xs)
                    bn6 = trtp.tile([128, 12], f32, tag="bn6")
                    nc.vector.bn_stats(bn6[:, 0:6], xs[:, 0:512])
                    nc.vector.bn_stats(bn6[:, 6:12], xs[:, 512:1024])
                    ms = trtp.tile([128, 2], f32, tag="ms")
                    nc.vector.bn_aggr(
                        ms[:], bn6[:].rearrange("p (a s) -> p a s", a=2))
                    lv = trtp.tile([128, 1], f32, tag="lv")
                    nc.scalar.activation(lv[:], ms[:, 1:2], AF.Ln,
                                         bias=epsv[:])
                    rs = trp.tile([128, 1], f32, tag="rstd2")
                    nc.scalar.activation(rs[:], lv[:], AF.Exp, scale=-0.5)
                    rstd2.append(rs)
                    for cb in range(NCB):
                        eng = nc.sync if cb % 2 == 0 else nc.scalar
                        eng.dma_start_transpose(
                            XT[:, cb * 1024 + tb * 128:
                               cb * 1024 + tb * 128 + 128],
                            X[:, tb * 1024 + cb * 128:
                              tb * 1024 + cb * 128 + 128])

                # load persistent-memory keys into the vnorm slot
                pktall = wp.tile([128, 4224], bf16, tag="vnorm")
                nc.gpsimd.dma_start(pktall[:, 0:4096], pkt_in[:])

                # -------- persistent branch --------
                branch(1, wkm_in, wrow["wkm"], dmatm_in, svecm, cc2_in,
                       cp_in=cpm_in, rstd_pre=rstd2)

    return nc


_prog_cache = {}


def _get_program():
    if "nc" not in _prog_cache:
        _prog_cache["nc"] = _build_program()
    return _prog_cache["nc"]


def _host_prep(inputs):
    """Build the 8 per-core input maps from the full-problem inputs."""
    x = np.asarray(inputs["x"], np.float32)
    ln1 = np.asarray(inputs["ln1_w"], np.float32)
    ln2 = np.asarray(inputs["ln2_w"], np.float32)
    Wk_a = np.asarray(inputs["Wk_a"], np.float32)
    Wv_a = np.asarray(inputs["Wv_a"], np.float32)
    cproj_a = np.asarray(inputs["cproj_a"], np.float32)
    beta_a = np.asarray(inputs["beta_a"], np.float32).reshape(NH)
    kscale_a = np.asarray(inputs["kscale_a"], np.float32).reshape(NH)
    vcoef = np.asarray(inputs["vcoef"], np.float32).reshape(NH)
    vscale = np.asarray(inputs["vscale"], np.float32).reshape(NH)
    Wk_m = np.asarray(inputs["Wk_m"], np.float32)
    beta_m = np.asarray(inputs["beta_m"], np.float32).reshape(NH)
    kscale_m = np.asarray(inputs["kscale_m"], np.float32).reshape(NH)
    Pk = np.asarray(inputs["Pk"], np.float32)
    Pv = np.asarray(inputs["Pv"], np.float32)
    out_scale = np.asarray(inputs["out_scale"], np.float32).reshape(NH)
    cproj_m = np.asarray(inputs["cproj_m"], np.float32)

    J, I = np.meshgrid(np.arange(128), np.arange(128), indexing="ij")

    def dmats(beta, heads):
        out = np.zeros((128, NHL * 256), np.float32)
        for i, h in enumerate(heads):
            b = abs(float(beta[h])) * EXP_SCALING
            out[:, i * 256:i * 256 + 128] = np.where(
                I >= J, np.exp(-(I - J) * b), 0.0)
            out[:, i * 256 + 128:i * 256 + 256] = np.exp(-((I + 128) - J) * b)
        return out

    # context diagonal masks: mask_r[jl, ql] = 1 if jl + r*128 < ql
    maskt = np.zeros((128, 2048), np.float32)
    jl = np.arange(128)[:, None]
    ql = np.arange(512)[None, :]
    for r in range(4):
        maskt[:, r * 512:(r + 1) * 512] = (jl + r * 128 < ql)

    vpad = np.zeros((128, 16), np.float32)
    vpad[:, 0::2] = 1.0

    def _smat():
        st = np.eye(128, k=-1, dtype=np.float32)   # S_T[j,t]=1 iff j==t+1
        e = np.zeros((128, 128), np.float32)
        e[0, 127] = 1.0                            # row127 <- next block row0
        return np.concatenate([st, e], axis=1)

    base = {
        "ident": np.eye(128, dtype=nbf),
        "maskt": maskt.astype(nbf),
        "ones1": np.ones((1, 64), nbf),
        "onesr": np.ones((1, 512), nbf),
        "onescol": np.ones((128, 1), nbf),
        "vpad": vpad.astype(nbf),
        "zcol": np.zeros((128, 1), nbf),
        "epsv": np.full((128, 1), 1e-5, np.float32),
        "smat": _smat().astype(nbf),
    }

    in_maps = []
    for c in range(N_CORES):
        b = c // 2
        hh = c % 2
        cols = slice(hh * CL, (hh + 1) * CL)
        heads = list(range(hh * NHL, hh * NHL + NHL))

        wka = (Wk_a * ln1[None, :])[cols].T.copy()      # (C, 512)
        wva = (Wv_a * ln1[None, :])[cols].T.copy()
        wkm = (Wk_m * ln2[None, :])[cols].T.copy()

        sva = np.exp(np.minimum(1.0 * EXP_SCALING * kscale_a[heads],
                                KSCALE_MAX))
        svm = np.exp(np.minimum(2.0 * EXP_SCALING * kscale_m[heads],
                                KSCALE_MAX))
        vs = np.exp(EXP_SCALING * vscale[heads])
        c1 = 1.0 - vcoef[heads]
        osc = np.exp(EXP_SCALING * out_scale[heads]) / Pk.shape[0]

        pkt = np.zeros((128, 4096), np.float32)
        pvo = np.zeros((128, NHL * 8 * 66), np.float32)
        for i, h in enumerate(heads):
            pb_ = (i % 2) * 64
            fb = (i // 2) * 1024
            pkt[pb_:pb_ + 64, fb:fb + 1024] = Pk[0, 0, h].T
            for pb2 in range(8):
                col = i * 528 + pb2 * 66
                pvo[:, col:col + 64] = Pv[0, 0, h, pb2 * 128:(pb2 + 1) * 128,
                                          :] * osc[i]
                pvo[:, col + 64] = 1.0
                pvo[:, col + 65] = 0.0

        xb = np.ascontiguousarray(x[b]).astype(nbf)
        coefb = np.repeat(c1, HS)[None, :].repeat(128, 0)

        m = dict(base)
        m.update({
            "x": xb,
            "xt": np.ascontiguousarray(xb.T),
            "wka": np.ascontiguousarray(wka).astype(nbf),
            "wva": np.ascontiguousarray(wva).astype(nbf),
            "wkm": np.ascontiguousarray(wkm).astype(nbf),
            "cpa": np.ascontiguousarray(cproj_a[:, cols].T).astype(nbf),
            "cpm": np.ascontiguousarray(cproj_m[:, cols].T).astype(nbf),
            "wkar": wka.sum(0, dtype=np.float64).astype(nbf)[None, :],
            "wvar": wva.sum(0, dtype=np.float64).astype(nbf)[None, :],
            "wkmr": wkm.sum(0, dtype=np.float64).astype(nbf)[None, :],
            "dmata": dmats(beta_a, heads).astype(nbf),
            "dmatm": dmats(beta_m, heads).astype(nbf),
            "sveca": np.broadcast_to(sva, (128, NHL)).astype(np.float32).copy(),
            "svecm": np.broadcast_to(svm, (128, NHL)).astype(np.float32).copy(),
            "coefb": coefb.astype(nbf),
            "vs": np.broadcast_to(vs, (128, NHL)).astype(np.float32).copy(),
            "pkt": pkt.astype(nbf),
            "pvo": pvo.astype(nbf),
        })
        in_maps.append(m)
    return in_maps


def _assemble(res):
    out = np.empty((B, T, C), np.float32)
    for b in range(B):
        out[b] = np.asarray(res.results[2 * b]["xp"], np.float32)
        pm0 = np.asarray(res.results[2 * b]["pm"], np.float32)
        pm1 = np.asarray(res.results[2 * b + 1]["pm"], np.float32)
        for t0, n, o in PM_CHUNKS:
            h = n // 2
            out[b, t0:t0 + h] += pm0[o:o + h]
            out[b, t0 + h:t0 + n] += pm1[o:o + h]
    return out


def kernel(**inputs):
    nc = _get_program()
    in_maps = _host_prep(inputs)
    res = run_bass_kernel_spmd(nc, in_maps, list(range(N_CORES)))
    return _assemble(res)


def kernel_traced(**inputs):
    """Like kernel() but returns (out, BassKernelResults) with HW timing."""
    nc = _get_program()
    in_maps = _host_prep(inputs)
    res = run_bass_kernel_spmd(nc, in_maps, list(range(N_CORES)), trace=True)
    return _assemble(res), res
